# revision 2
# baseline (speedup 1.0000x reference)
"""Trainium2 Bass kernel for nn_Mlp_2_Layer (moe_routing) — v3.

Data-parallel over batch (1024 samples/core, all 8 domains/core), with
the three big baseline losses fixed:

1. No L1 recompute: the L1 pre-activations are copied PSUM->SBUF bf16
   (128KB/partition tile) and re-read for the ReLU; the baseline ran
   the 512 L1 matmuls twice.
2. Less collective stall: the BN-stats AllReduces are split into
   domain halves and kicked as soon as each half's partials are
   ready, so most of their latency hides under remaining compute.
   (A collective blocks the Pool dynamic-DMA ring until it completes,
   so they can only sit after all the gathers in the Pool queue.)
3. No PE time on transposes / gather-chasing head: the embedding table
   is bf16, gathered per (feature, batch-tile) with indirect DMA
   (Pool), transposed feature-major by the DMA xbar
   (dma_start_transpose), and L1 runs in bf16 at N=256 granularity so
   the tensor engine chases the gather stream; bf16 matmuls run at
   full PE rate (1 row/cycle) like f32r but halve SBUF/DMA.

h2 (the L2 pre-activations, needed again after the BN2 stats
AllReduce) is written into the SBUF space of pre1 that the same
(domain, chunk) just consumed — no HBM spill.

Final pass (per domain): a2 = relu(s2*h2+t2), W3-column matmuls reduce
over the 512 h2 partitions, sigmoid(+b3). P3 for domains 0-3 is
interleaved with L2 of domains 4-7 to hide the second stats AllReduce.

Host combines: final[b] = out[domain_id[b], b].
"""
import sys

for _p in ("/opt/trn_rl_repo", "/root/.axon_site"):
    if _p not in sys.path:
        sys.path.insert(0, _p)

import numpy as np

B, F, E, V = 8192, 16, 32, 100000
D, H1, H2 = 8, 1024, 512
IN = F * E          # 512
EPS = 1e-5
NCORES = 8
BC = B // NCORES    # 1024 samples per core
P = 128
NBT = BC // P       # 8 batch tiles per core
NBP = NBT // 2      # 4 batch-tile pairs (N=256 L1 chunks)
NT = BC // 512      # 2 n-chunks of 512 per core
K1 = IN // P        # 4
M1 = H1 // P        # 8
K2 = H1 // P        # 8
M2 = H2 // P        # 4
HD = D // 2         # stats collectives split into two domain halves

PROFILE = False
LAST_EXEC_NS = None

_NC = None


def _build():
    import concourse.bass as bass
    import concourse.tile as tile
    from concourse import bacc, mybir
    from contextlib import ExitStack

    f32 = mybir.dt.float32
    bf16 = mybir.dt.bfloat16
    i32 = mybir.dt.int32
    AF = mybir.ActivationFunctionType
    ALU = mybir.AluOpType

    nc = bacc.Bacc(None, target_bir_lowering=False, debug=False)

    tab_d = nc.dram_tensor("tab", [F * V, E], bf16, kind="ExternalInput")
    gidx_d = nc.dram_tensor("gidx", [P, NBT * F], i32, kind="ExternalInput")
    w1t_d = nc.dram_tensor("w1t", [D, P, K1, H1], bf16, kind="ExternalInput")
    w2t_d = nc.dram_tensor("w2t", [D, P, K2, H2], bf16, kind="ExternalInput")
    g1_d = nc.dram_tensor("g1", [D, H1], f32, kind="ExternalInput")
    be1_d = nc.dram_tensor("be1", [D, H1], f32, kind="ExternalInput")
    g2_d = nc.dram_tensor("g2", [D, H2], f32, kind="ExternalInput")
    be2_d = nc.dram_tensor("be2", [D, H2], f32, kind="ExternalInput")
    w3_d = nc.dram_tensor("w3", [D, H2], bf16, kind="ExternalInput")
    b3_d = nc.dram_tensor("b3p", [1, D], f32, kind="ExternalInput")
    out_d = nc.dram_tensor("out", [D, BC], f32, kind="ExternalOutput")

    # collectives: 2 halves x 2 layers (NO warmup AR: a collective
    # blocks the Pool qPoolDynamic ring until it completes, so a warmup
    # at t=0 just stalls the gathers that feed everything)
    n1 = 2 * HD * M1
    n2 = 2 * HD * M2
    cc_sizes = [n1, n1, n2, n2]
    cc_in = [nc.dram_tensor(f"cci{i}", [P, s], f32, kind="Internal")
             for i, s in enumerate(cc_sizes)]
    cc_out = [nc.dram_tensor(f"cco{i}", [P, s], f32, kind="Internal",
                             addr_space="Shared")
              for i, s in enumerate(cc_sizes)]
    RG = [list(range(NCORES))]

    with tile.TileContext(nc) as tc:
        with ExitStack() as ctx:
            const = ctx.enter_context(tc.tile_pool(name="const", bufs=1))
            stp = ctx.enter_context(tc.tile_pool(name="stp", bufs=1))
            gpool = ctx.enter_context(tc.tile_pool(name="gpool", bufs=4))
            wpool = ctx.enter_context(tc.tile_pool(name="wpool", bufs=2))
            a1p = ctx.enter_context(tc.tile_pool(name="a1p", bufs=2))
            a2p = ctx.enter_context(tc.tile_pool(name="a2p", bufs=2))
            sgp = ctx.enter_context(tc.tile_pool(name="sgp", bufs=2))
            l1ps = ctx.enter_context(tc.tile_pool(name="l1ps", bufs=4, space="PSUM"))
            l2ps = ctx.enter_context(tc.tile_pool(name="l2ps", bufs=3, space="PSUM"))
            w3ps = ctx.enter_context(tc.tile_pool(name="w3ps", bufs=1, space="PSUM"))

            # ---- consts ----
            eps_t = const.tile([P, 1], f32)
            nc.vector.memset(eps_t[:], EPS)

            gidx = const.tile([P, NBT * F], i32)
            nc.sync.dma_start(out=gidx[:], in_=gidx_d[:, :])
            g1c = const.tile([P, D * M1], f32)
            nc.sync.dma_start(out=g1c[:], in_=g1_d[:, :].rearrange(
                "d (m p) -> p (d m)", p=P))
            be1c = const.tile([P, D * M1], f32)
            nc.sync.dma_start(out=be1c[:], in_=be1_d[:, :].rearrange(
                "d (m p) -> p (d m)", p=P))
            g2c = const.tile([P, D * M2], f32)
            nc.sync.dma_start(out=g2c[:], in_=g2_d[:, :].rearrange(
                "d (m p) -> p (d m)", p=P))
            be2c = const.tile([P, D * M2], f32)
            nc.sync.dma_start(out=be2c[:], in_=be2_d[:, :].rearrange(
                "d (m p) -> p (d m)", p=P))
            w3c = const.tile([P, D * M2], bf16)
            nc.sync.dma_start(out=w3c[:], in_=w3_d[:, :].rearrange(
                "d (m p) -> p (d m)", p=P))
            b3p = const.tile([1, D], f32)
            nc.sync.dma_start(out=b3p[:], in_=b3_d[:, :])

            # ---- persistent state ----
            xt = stp.tile([P, K1, BC], bf16, name="xt")
            pre1 = stp.tile([P, D, M1, BC], bf16, name="pre1")
            st1 = [stp.tile([P, HD, M1, NBP, 6], f32, name=f"st1_{h}")
                   for h in range(2)]
            mv1 = [stp.tile([P, HD, M1, 2], f32, name=f"mv1_{h}")
                   for h in range(2)]
            st2 = [stp.tile([P, HD, M2, NT, 6], f32, name=f"st2_{h}")
                   for h in range(2)]
            mv2 = [stp.tile([P, HD, M2, 2], f32, name=f"mv2_{h}")
                   for h in range(2)]
            s1 = [stp.tile([P, HD * M1], f32, name=f"s1_{h}") for h in range(2)]
            t1 = [stp.tile([P, HD * M1], f32, name=f"t1_{h}") for h in range(2)]
            s2 = [stp.tile([P, HD * M2], f32, name=f"s2_{h}") for h in range(2)]
            t2 = [stp.tile([P, HD * M2], f32, name=f"t2_{h}") for h in range(2)]
            uq1 = [stp.tile([P, n1], f32, name=f"uq1_{h}") for h in range(2)]
            sa1 = [stp.tile([P, n1], f32, name=f"sa1_{h}") for h in range(2)]
            uq2 = [stp.tile([P, n2], f32, name=f"uq2_{h}") for h in range(2)]
            sa2 = [stp.tile([P, n2], f32, name=f"sa2_{h}") for h in range(2)]
            tmps = [stp.tile([P, HD * M1], f32, name=f"tmp_{i}")
                    for i in range(4)]

            def stats_pack(h, M, st, mv, uq, cci, cco):
                """Per-core (mean, E[x^2]) for domain half h -> AllReduce."""
                for dd in range(HD):
                    for m in range(M):
                        nc.vector.bn_aggr(out=mv[:, dd, m, :],
                                          in_=st[:, dd, m, :, :])
                n = HD * M
                u = uq[:, 0:n].rearrange("p (d m) -> p d m", d=HD)
                q = uq[:, n:].rearrange("p (d m) -> p d m", d=HD)
                nc.vector.tensor_copy(out=u, in_=mv[:, :, :, 0])
                nc.vector.tensor_mul(out=q, in0=mv[:, :, :, 0],
                                     in1=mv[:, :, :, 0])
                nc.vector.tensor_add(out=q, in0=q, in1=mv[:, :, :, 1])
                nc.gpsimd.dma_start(out=cci[:, :], in_=uq[:])
                nc.gpsimd.collective_compute(
                    "AllReduce", ALU.add, replica_groups=RG,
                    ins=[cci[:, :]], outs=[cco[:, :]])

            def stats_apply(idx, h, M, sa, cco, g_c, be_c, s_t, t_t):
                nc.gpsimd.dma_start(out=sa[:], in_=cco[:, :])
                n = HD * M
                lo = h * HD
                mean = tmps[idx][:, 0:n]
                var = sa[:, n:]
                nc.vector.tensor_scalar_mul(mean, sa[:, 0:n], 1.0 / NCORES)
                nc.vector.tensor_scalar_mul(var, var, 1.0 / NCORES)
                gl = slice(lo * M, (lo + HD) * M)
                nc.vector.tensor_mul(out=s_t[:], in0=mean, in1=mean)
                nc.vector.tensor_tensor(out=var, in0=var, in1=s_t[:],
                                        op=ALU.subtract)
                nc.scalar.activation(out=var, in_=var, func=AF.Sqrt,
                                     bias=eps_t[:], scale=1.0)
                nc.vector.reciprocal(out=var, in_=var)
                nc.vector.tensor_mul(out=s_t[:], in0=g_c[:, gl], in1=var)
                nc.vector.tensor_mul(out=t_t[:], in0=mean, in1=s_t[:])
                nc.vector.tensor_tensor(out=t_t[:], in0=be_c[:, gl],
                                        in1=t_t[:], op=ALU.subtract)

            # ---- P1: gather + xbar-transpose + L1 (N=256, chases gather) ----
            for bp in range(NBP):
                for bt2 in range(2):
                    bt = 2 * bp + bt2
                    G = gpool.tile([P, F, E], bf16, tag="G")
                    for f in range(F):
                        nc.gpsimd.indirect_dma_start(
                            out=G[:, f, :],
                            out_offset=None,
                            in_=tab_d[:, :],
                            in_offset=bass.IndirectOffsetOnAxis(
                                ap=gidx[:, bt * F + f: bt * F + f + 1],
                                axis=0),
                        )
                    for k in range(K1):
                        nc.sync.dma_start_transpose(
                            out=xt[:, k, bt * P:(bt + 1) * P],
                            in_=G[:, 4 * k:4 * (k + 1), :])
                for d in range(D):
                    w1 = wpool.tile([P, K1, H1], bf16, tag="w")
                    nc.sync.dma_start(out=w1[:], in_=w1t_d[d, :, :, :])
                    csl = slice(bp * 256, (bp + 1) * 256)
                    for m in range(M1):
                        pm = l1ps.tile([P, 256], f32, tag="pm")
                        for k in range(K1):
                            nc.tensor.matmul(
                                out=pm[:],
                                lhsT=w1[:, k, m * P:(m + 1) * P],
                                rhs=xt[:, k, csl],
                                start=(k == 0), stop=(k == K1 - 1))
                        nc.vector.bn_stats(
                            out=st1[d // HD][:, d % HD, m, bp, :], in_=pm[:])
                        nc.scalar.copy(out=pre1[:, d, m, csl], in_=pm[:])
                    if bp == NBP - 1 and d == HD - 1:
                        stats_pack(0, M1, st1[0], mv1[0], uq1[0],
                                   cc_in[0], cc_out[0])
            stats_pack(1, M1, st1[1], mv1[1], uq1[1], cc_in[1], cc_out[1])
            stats_apply(0, 0, M1, sa1[0], cc_out[0], g1c, be1c, s1[0], t1[0])

            # ---- P2: a1 = relu(s1*pre1+t1); L2; h2 into pre1's dead space --
            def p3_domain(d):
                hh = d // HD
                for nt in range(NT):
                    nsl = slice(nt * 512, (nt + 1) * 512)
                    a2 = a2p.tile([P, M2, 512], bf16, tag="a2")
                    po = w3ps.tile([1, 512], f32, tag="po")
                    for m2 in range(M2):
                        dm = (d % HD) * M2 + m2
                        nc.scalar.activation(
                            out=a2[:, m2, :], in_=pre1[:, d, m2, nsl],
                            func=AF.Relu,
                            bias=t2[hh][:, dm:dm + 1],
                            scale=s2[hh][:, dm:dm + 1])
                        nc.tensor.matmul(
                            out=po[:],
                            lhsT=w3c[:, d * M2 + m2:d * M2 + m2 + 1],
                            rhs=a2[:, m2, :],
                            start=(m2 == 0), stop=(m2 == M2 - 1))
                    sg = sgp.tile([1, 512], f32, tag="sg")
                    nc.scalar.activation(out=sg[:], in_=po[:], func=AF.Sigmoid,
                                         bias=b3p[0:1, d:d + 1], scale=1.0)
                    nc.sync.dma_start(out=out_d[d, nsl], in_=sg[:])

            for d in range(D):
                hh = d // HD
                if d == HD:
                    stats_apply(1, 1, M1, sa1[1], cc_out[1], g1c, be1c,
                                s1[1], t1[1])
                w2 = wpool.tile([P, K2, H2], bf16, tag="w")
                nc.sync.dma_start(out=w2[:], in_=w2t_d[d, :, :, :])
                for nt in range(NT):
                    nsl = slice(nt * 512, (nt + 1) * 512)
                    a1 = a1p.tile([P, K2, 512], bf16, tag="a1")
                    for m in range(M1):
                        dm = (d % HD) * M1 + m
                        nc.scalar.activation(
                            out=a1[:, m, :], in_=pre1[:, d, m, nsl],
                            func=AF.Relu,
                            bias=t1[hh][:, dm:dm + 1],
                            scale=s1[hh][:, dm:dm + 1])
                    for m2 in range(M2):
                        pm2 = l2ps.tile([P, 512], f32, tag="pm2")
                        for k2 in range(K2):
                            nc.tensor.matmul(
                                out=pm2[:],
                                lhsT=w2[:, k2, m2 * P:(m2 + 1) * P],
                                rhs=a1[:, k2, :],
                                start=(k2 == 0), stop=(k2 == K2 - 1))
                        nc.vector.bn_stats(
                            out=st2[d // HD][:, d % HD, m2, nt, :],
                            in_=pm2[:])
                        # h2 reuses the pre1 slot this (d, nt) just consumed
                        nc.vector.tensor_copy(out=pre1[:, d, m2, nsl],
                                              in_=pm2[:])
                if d == HD - 1:
                    stats_pack(0, M2, st2[0], mv2[0], uq2[0],
                               cc_in[2], cc_out[2])
                if d == HD:
                    stats_apply(2, 0, M2, sa2[0], cc_out[2], g2c, be2c,
                                s2[0], t2[0])
                if d >= HD:
                    # interleave P3 of domain (d-HD) under L2 of domain d
                    p3_domain(d - HD)
            stats_pack(1, M2, st2[1], mv2[1], uq2[1], cc_in[3], cc_out[3])
            stats_apply(3, 1, M2, sa2[1], cc_out[3], g2c, be2c, s2[1], t2[1])
            for d in range(HD, D):
                p3_domain(d)

    nc.compile()
    return nc


def kernel(**inputs):
    global _NC, LAST_EXEC_NS
    from concourse.bass_utils import run_bass_kernel_spmd
    import ml_dtypes

    bf = ml_dtypes.bfloat16

    feat_ids = np.asarray(inputs["feat_ids"])
    domain_id = np.asarray(inputs["domain_id"])
    emb_tables = np.asarray(inputs["emb_tables"], dtype=np.float32)
    W1 = np.asarray(inputs["W1"], dtype=np.float32)
    g1 = np.asarray(inputs["g1"], dtype=np.float32)
    be1 = np.asarray(inputs["be1"], dtype=np.float32)
    W2 = np.asarray(inputs["W2"], dtype=np.float32)
    g2 = np.asarray(inputs["g2"], dtype=np.float32)
    be2 = np.asarray(inputs["be2"], dtype=np.float32)
    W3 = np.asarray(inputs["W3"], dtype=np.float32)
    b3 = np.asarray(inputs["b3"], dtype=np.float32)
    # b1/b2 cancel inside training-mode BatchNorm (constant shift).

    if _NC is None:
        _NC = _build()

    tab = np.ascontiguousarray(emb_tables.reshape(F * V, E).astype(bf))
    w1t = np.ascontiguousarray(
        W1.transpose(0, 2, 1).reshape(D, K1, P, H1)
        .transpose(0, 2, 1, 3).astype(bf))            # [D, P, K1, H1]
    w2t = np.ascontiguousarray(
        W2.transpose(0, 2, 1).reshape(D, K2, P, H2)
        .transpose(0, 2, 1, 3).astype(bf))            # [D, P, K2, H2]
    b3p = np.ascontiguousarray(b3.reshape(1, D))

    ids = feat_ids.astype(np.int64)
    in_maps = []
    for c in range(NCORES):
        idc = ids[c * BC:(c + 1) * BC]                # [BC, F]
        g = idc.reshape(NBT, P, F).transpose(1, 0, 2)
        g = g + (np.arange(F, dtype=np.int64) * V)[None, None, :]
        gidx = np.ascontiguousarray(g.reshape(P, NBT * F).astype(np.int32))
        in_maps.append({
            "tab": tab, "gidx": gidx,
            "w1t": w1t, "w2t": w2t,
            "g1": g1, "be1": be1, "g2": g2, "be2": be2,
            "w3": W3.astype(bf), "b3p": b3p,
        })

    res = run_bass_kernel_spmd(
        _NC, in_maps, core_ids=list(range(NCORES)), trace=bool(PROFILE))
    if PROFILE:
        LAST_EXEC_NS = res.exec_time_ns

    out_full = np.concatenate(
        [res.results[c]["out"] for c in range(NCORES)], axis=1)  # [D, B]
    final = out_full[domain_id.astype(np.int64), np.arange(B)]
    return final.astype(np.float32)


# revision 3
# speedup vs baseline: 1.0262x; 1.0262x over previous
"""Trainium2 Bass kernel for nn_Mlp_2_Layer (moe_routing) — v3.

Data-parallel over batch (1024 samples/core, all 8 domains/core), with
the three big baseline losses fixed:

1. No L1 recompute: the L1 pre-activations are copied PSUM->SBUF bf16
   (128KB/partition tile) and re-read for the ReLU; the baseline ran
   the 512 L1 matmuls twice.
2. Less collective stall: the BN-stats AllReduces are split into
   domain halves and kicked as soon as each half's partials are
   ready, so most of their latency hides under remaining compute.
   (A collective blocks the Pool dynamic-DMA ring until it completes,
   so they can only sit after all the gathers in the Pool queue.)
3. No PE time on transposes / gather-chasing head: the embedding table
   is bf16, gathered per (feature, batch-tile) with indirect DMA
   (Pool), transposed feature-major by the DMA xbar
   (dma_start_transpose), and L1 runs in bf16 at N=256 granularity so
   the tensor engine chases the gather stream; bf16 matmuls run at
   full PE rate (1 row/cycle) like f32r but halve SBUF/DMA.

h2 (the L2 pre-activations, needed again after the BN2 stats
AllReduce) is written into the SBUF space of pre1 that the same
(domain, chunk) just consumed — no HBM spill.

Final pass (per domain): a2 = relu(s2*h2+t2), W3-column matmuls reduce
over the 512 h2 partitions, sigmoid(+b3). P3 for domains 0-3 is
interleaved with L2 of domains 4-7 to hide the second stats AllReduce.

Host combines: final[b] = out[domain_id[b], b].
"""
import sys

for _p in ("/opt/trn_rl_repo", "/root/.axon_site"):
    if _p not in sys.path:
        sys.path.insert(0, _p)

import numpy as np

B, F, E, V = 8192, 16, 32, 100000
D, H1, H2 = 8, 1024, 512
IN = F * E          # 512
EPS = 1e-5
NCORES = 8
BC = B // NCORES    # 1024 samples per core
P = 128
NBT = BC // P       # 8 batch tiles per core
NBP = NBT // 2      # 4 batch-tile pairs (N=256 L1 chunks)
NT = BC // 512      # 2 n-chunks of 512 per core
K1 = IN // P        # 4
M1 = H1 // P        # 8
K2 = H1 // P        # 8
M2 = H2 // P        # 4
HD = D // 2         # stats collectives split into two domain halves

PROFILE = False
LAST_EXEC_NS = None

_NC = None


def _build():
    import concourse.bass as bass
    import concourse.tile as tile
    from concourse import bacc, mybir
    from concourse.masks import make_identity
    from contextlib import ExitStack

    f32 = mybir.dt.float32
    bf16 = mybir.dt.bfloat16
    i32 = mybir.dt.int32
    AF = mybir.ActivationFunctionType
    ALU = mybir.AluOpType

    nc = bacc.Bacc(None, target_bir_lowering=False, debug=False)

    tab_d = nc.dram_tensor("tab", [F * V, E], bf16, kind="ExternalInput")
    gidx_d = nc.dram_tensor("gidx", [P, NBT * F], i32, kind="ExternalInput")
    w1t_d = nc.dram_tensor("w1t", [D, P, K1, H1], bf16, kind="ExternalInput")
    w2t_d = nc.dram_tensor("w2t", [D, P, K2, H2], bf16, kind="ExternalInput")
    g1_d = nc.dram_tensor("g1", [D, H1], f32, kind="ExternalInput")
    be1_d = nc.dram_tensor("be1", [D, H1], f32, kind="ExternalInput")
    g2_d = nc.dram_tensor("g2", [D, H2], f32, kind="ExternalInput")
    be2_d = nc.dram_tensor("be2", [D, H2], f32, kind="ExternalInput")
    w3_d = nc.dram_tensor("w3", [D, H2], bf16, kind="ExternalInput")
    b3_d = nc.dram_tensor("b3p", [1, D], f32, kind="ExternalInput")
    out_d = nc.dram_tensor("out", [D, BC], f32, kind="ExternalOutput")

    # collectives: 2 halves x 2 layers (NO warmup AR: a collective
    # blocks the Pool qPoolDynamic ring until it completes, so a warmup
    # at t=0 just stalls the gathers that feed everything)
    n1 = 2 * HD * M1
    n2 = 2 * HD * M2
    cc_sizes = [n1, n1, n2, n2]
    cc_in = [nc.dram_tensor(f"cci{i}", [P, s], f32, kind="Internal")
             for i, s in enumerate(cc_sizes)]
    cc_out = [nc.dram_tensor(f"cco{i}", [P, s], f32, kind="Internal",
                             addr_space="Shared")
              for i, s in enumerate(cc_sizes)]
    RG = [list(range(NCORES))]

    with tile.TileContext(nc) as tc:
        with ExitStack() as ctx:
            const = ctx.enter_context(tc.tile_pool(name="const", bufs=1))
            stp = ctx.enter_context(tc.tile_pool(name="stp", bufs=1))
            gpool = ctx.enter_context(tc.tile_pool(name="gpool", bufs=4))
            wpool = ctx.enter_context(tc.tile_pool(name="wpool", bufs=2))
            a1p = ctx.enter_context(tc.tile_pool(name="a1p", bufs=2))
            a2p = ctx.enter_context(tc.tile_pool(name="a2p", bufs=2))
            sgp = ctx.enter_context(tc.tile_pool(name="sgp", bufs=2))
            l1ps = ctx.enter_context(tc.tile_pool(name="l1ps", bufs=3, space="PSUM"))
            l2ps = ctx.enter_context(tc.tile_pool(name="l2ps", bufs=3, space="PSUM"))
            w3ps = ctx.enter_context(tc.tile_pool(name="w3ps", bufs=1, space="PSUM"))
            tpps = ctx.enter_context(tc.tile_pool(name="tpps", bufs=1, space="PSUM"))

            # ---- consts ----
            eps_t = const.tile([P, 1], f32)
            nc.vector.memset(eps_t[:], EPS)
            ident = const.tile([P, P], bf16)
            make_identity(nc, ident[:])

            gidx = const.tile([P, NBT * F], i32)
            nc.sync.dma_start(out=gidx[:], in_=gidx_d[:, :])
            g1c = const.tile([P, D * M1], f32)
            nc.sync.dma_start(out=g1c[:], in_=g1_d[:, :].rearrange(
                "d (m p) -> p (d m)", p=P))
            be1c = const.tile([P, D * M1], f32)
            nc.sync.dma_start(out=be1c[:], in_=be1_d[:, :].rearrange(
                "d (m p) -> p (d m)", p=P))
            g2c = const.tile([P, D * M2], f32)
            nc.sync.dma_start(out=g2c[:], in_=g2_d[:, :].rearrange(
                "d (m p) -> p (d m)", p=P))
            be2c = const.tile([P, D * M2], f32)
            nc.sync.dma_start(out=be2c[:], in_=be2_d[:, :].rearrange(
                "d (m p) -> p (d m)", p=P))
            w3c = const.tile([P, D * M2], bf16)
            nc.sync.dma_start(out=w3c[:], in_=w3_d[:, :].rearrange(
                "d (m p) -> p (d m)", p=P))
            b3p = const.tile([1, D], f32)
            nc.sync.dma_start(out=b3p[:], in_=b3_d[:, :])

            # ---- persistent state ----
            xt = stp.tile([P, K1, BC], bf16, name="xt")
            pre1 = stp.tile([P, D, M1, BC], bf16, name="pre1")
            st1 = [stp.tile([P, HD, M1, NBP, 6], f32, name=f"st1_{h}")
                   for h in range(2)]
            mv1 = [stp.tile([P, HD, M1, 2], f32, name=f"mv1_{h}")
                   for h in range(2)]
            st2 = [stp.tile([P, HD, M2, NT, 6], f32, name=f"st2_{h}")
                   for h in range(2)]
            mv2 = [stp.tile([P, HD, M2, 2], f32, name=f"mv2_{h}")
                   for h in range(2)]
            s1 = [stp.tile([P, HD * M1], f32, name=f"s1_{h}") for h in range(2)]
            t1 = [stp.tile([P, HD * M1], f32, name=f"t1_{h}") for h in range(2)]
            s2 = [stp.tile([P, HD * M2], f32, name=f"s2_{h}") for h in range(2)]
            t2 = [stp.tile([P, HD * M2], f32, name=f"t2_{h}") for h in range(2)]
            uq1 = [stp.tile([P, n1], f32, name=f"uq1_{h}") for h in range(2)]
            sa1 = [stp.tile([P, n1], f32, name=f"sa1_{h}") for h in range(2)]
            uq2 = [stp.tile([P, n2], f32, name=f"uq2_{h}") for h in range(2)]
            sa2 = [stp.tile([P, n2], f32, name=f"sa2_{h}") for h in range(2)]
            tmps = [stp.tile([P, HD * M1], f32, name=f"tmp_{i}")
                    for i in range(4)]

            def stats_pack(h, M, st, mv, uq, cci, cco):
                """Per-core (mean, E[x^2]) for domain half h -> AllReduce."""
                for dd in range(HD):
                    for m in range(M):
                        nc.vector.bn_aggr(out=mv[:, dd, m, :],
                                          in_=st[:, dd, m, :, :])
                n = HD * M
                u = uq[:, 0:n].rearrange("p (d m) -> p d m", d=HD)
                q = uq[:, n:].rearrange("p (d m) -> p d m", d=HD)
                nc.vector.tensor_copy(out=u, in_=mv[:, :, :, 0])
                nc.vector.tensor_mul(out=q, in0=mv[:, :, :, 0],
                                     in1=mv[:, :, :, 0])
                nc.vector.tensor_add(out=q, in0=q, in1=mv[:, :, :, 1])
                nc.gpsimd.dma_start(out=cci[:, :], in_=uq[:])
                nc.gpsimd.collective_compute(
                    "AllReduce", ALU.add, replica_groups=RG,
                    ins=[cci[:, :]], outs=[cco[:, :]])

            def stats_apply(idx, h, M, sa, cco, g_c, be_c, s_t, t_t):
                nc.gpsimd.dma_start(out=sa[:], in_=cco[:, :])
                n = HD * M
                lo = h * HD
                mean = tmps[idx][:, 0:n]
                var = sa[:, n:]
                nc.vector.tensor_scalar_mul(mean, sa[:, 0:n], 1.0 / NCORES)
                nc.vector.tensor_scalar_mul(var, var, 1.0 / NCORES)
                gl = slice(lo * M, (lo + HD) * M)
                nc.vector.tensor_mul(out=s_t[:], in0=mean, in1=mean)
                nc.vector.tensor_tensor(out=var, in0=var, in1=s_t[:],
                                        op=ALU.subtract)
                nc.scalar.activation(out=var, in_=var, func=AF.Sqrt,
                                     bias=eps_t[:], scale=1.0)
                nc.vector.reciprocal(out=var, in_=var)
                nc.vector.tensor_mul(out=s_t[:], in0=g_c[:, gl], in1=var)
                nc.vector.tensor_mul(out=t_t[:], in0=mean, in1=s_t[:])
                nc.vector.tensor_tensor(out=t_t[:], in0=be_c[:, gl],
                                        in1=t_t[:], op=ALU.subtract)

            # ---- P1: gather + xbar-transpose + L1 (N=256, chases gather) ----
            for bp in range(NBP):
                for bt2 in range(2):
                    bt = 2 * bp + bt2
                    G = gpool.tile([P, F, E], bf16, tag="G")
                    for f in range(F):
                        nc.gpsimd.indirect_dma_start(
                            out=G[:, f, :],
                            out_offset=None,
                            in_=tab_d[:, :],
                            in_offset=bass.IndirectOffsetOnAxis(
                                ap=gidx[:, bt * F + f: bt * F + f + 1],
                                axis=0),
                        )
                    gf = G[:].rearrange("p f e -> p (f e)")
                    tp = tpps.tile([P, K1 * P], bf16, tag="tp")
                    for k in range(K1):
                        nc.tensor.transpose(
                            out=tp[:, k * P:(k + 1) * P],
                            in_=gf[:, k * P:(k + 1) * P],
                            identity=ident[:])
                    nc.vector.tensor_copy(
                        out=xt[:, :, bt * P:(bt + 1) * P],
                        in_=tp[:].rearrange("p (k c) -> p k c", k=K1))
                for d in range(D):
                    w1 = wpool.tile([P, K1, H1], bf16, tag="w")
                    nc.sync.dma_start(out=w1[:], in_=w1t_d[d, :, :, :])
                    csl = slice(bp * 256, (bp + 1) * 256)
                    for m in range(M1):
                        pm = l1ps.tile([P, 256], f32, tag="pm")
                        for k in range(K1):
                            nc.tensor.matmul(
                                out=pm[:],
                                lhsT=w1[:, k, m * P:(m + 1) * P],
                                rhs=xt[:, k, csl],
                                start=(k == 0), stop=(k == K1 - 1))
                        nc.vector.bn_stats(
                            out=st1[d // HD][:, d % HD, m, bp, :], in_=pm[:])
                        nc.scalar.copy(out=pre1[:, d, m, csl], in_=pm[:])
                    if bp == NBP - 1 and d == HD - 1:
                        stats_pack(0, M1, st1[0], mv1[0], uq1[0],
                                   cc_in[0], cc_out[0])
            stats_pack(1, M1, st1[1], mv1[1], uq1[1], cc_in[1], cc_out[1])
            stats_apply(0, 0, M1, sa1[0], cc_out[0], g1c, be1c, s1[0], t1[0])

            # ---- P2: a1 = relu(s1*pre1+t1); L2; h2 into pre1's dead space --
            def p3_domain(d):
                hh = d // HD
                for nt in range(NT):
                    nsl = slice(nt * 512, (nt + 1) * 512)
                    a2 = a2p.tile([P, M2, 512], bf16, tag="a2")
                    po = w3ps.tile([1, 512], f32, tag="po")
                    for m2 in range(M2):
                        dm = (d % HD) * M2 + m2
                        nc.scalar.activation(
                            out=a2[:, m2, :], in_=pre1[:, d, m2, nsl],
                            func=AF.Relu,
                            bias=t2[hh][:, dm:dm + 1],
                            scale=s2[hh][:, dm:dm + 1])
                        nc.tensor.matmul(
                            out=po[:],
                            lhsT=w3c[:, d * M2 + m2:d * M2 + m2 + 1],
                            rhs=a2[:, m2, :],
                            start=(m2 == 0), stop=(m2 == M2 - 1))
                    sg = sgp.tile([1, 512], f32, tag="sg")
                    nc.scalar.activation(out=sg[:], in_=po[:], func=AF.Sigmoid,
                                         bias=b3p[0:1, d:d + 1], scale=1.0)
                    nc.sync.dma_start(out=out_d[d, nsl], in_=sg[:])

            for d in range(D):
                hh = d // HD
                if d == HD:
                    stats_apply(1, 1, M1, sa1[1], cc_out[1], g1c, be1c,
                                s1[1], t1[1])
                w2 = wpool.tile([P, K2, H2], bf16, tag="w")
                nc.sync.dma_start(out=w2[:], in_=w2t_d[d, :, :, :])
                for nt in range(NT):
                    nsl = slice(nt * 512, (nt + 1) * 512)
                    a1 = a1p.tile([P, K2, 512], bf16, tag="a1")
                    for m in range(M1):
                        dm = (d % HD) * M1 + m
                        nc.scalar.activation(
                            out=a1[:, m, :], in_=pre1[:, d, m, nsl],
                            func=AF.Relu,
                            bias=t1[hh][:, dm:dm + 1],
                            scale=s1[hh][:, dm:dm + 1])
                    for m2 in range(M2):
                        pm2 = l2ps.tile([P, 512], f32, tag="pm2")
                        for k2 in range(K2):
                            nc.tensor.matmul(
                                out=pm2[:],
                                lhsT=w2[:, k2, m2 * P:(m2 + 1) * P],
                                rhs=a1[:, k2, :],
                                start=(k2 == 0), stop=(k2 == K2 - 1))
                        nc.vector.bn_stats(
                            out=st2[d // HD][:, d % HD, m2, nt, :],
                            in_=pm2[:])
                        # h2 reuses the pre1 slot this (d, nt) just consumed
                        nc.vector.tensor_copy(out=pre1[:, d, m2, nsl],
                                              in_=pm2[:])
                if d == HD - 1:
                    stats_pack(0, M2, st2[0], mv2[0], uq2[0],
                               cc_in[2], cc_out[2])
                if d == HD:
                    stats_apply(2, 0, M2, sa2[0], cc_out[2], g2c, be2c,
                                s2[0], t2[0])
                if d >= HD:
                    # interleave P3 of domain (d-HD) under L2 of domain d
                    p3_domain(d - HD)
            stats_pack(1, M2, st2[1], mv2[1], uq2[1], cc_in[3], cc_out[3])
            stats_apply(3, 1, M2, sa2[1], cc_out[3], g2c, be2c, s2[1], t2[1])
            for d in range(HD, D):
                p3_domain(d)

    nc.compile()
    return nc


def kernel(**inputs):
    global _NC, LAST_EXEC_NS
    from concourse.bass_utils import run_bass_kernel_spmd
    import ml_dtypes

    bf = ml_dtypes.bfloat16

    feat_ids = np.asarray(inputs["feat_ids"])
    domain_id = np.asarray(inputs["domain_id"])
    emb_tables = np.asarray(inputs["emb_tables"], dtype=np.float32)
    W1 = np.asarray(inputs["W1"], dtype=np.float32)
    g1 = np.asarray(inputs["g1"], dtype=np.float32)
    be1 = np.asarray(inputs["be1"], dtype=np.float32)
    W2 = np.asarray(inputs["W2"], dtype=np.float32)
    g2 = np.asarray(inputs["g2"], dtype=np.float32)
    be2 = np.asarray(inputs["be2"], dtype=np.float32)
    W3 = np.asarray(inputs["W3"], dtype=np.float32)
    b3 = np.asarray(inputs["b3"], dtype=np.float32)
    # b1/b2 cancel inside training-mode BatchNorm (constant shift).

    if _NC is None:
        _NC = _build()

    tab = np.ascontiguousarray(emb_tables.reshape(F * V, E).astype(bf))
    w1t = np.ascontiguousarray(
        W1.transpose(0, 2, 1).reshape(D, K1, P, H1)
        .transpose(0, 2, 1, 3).astype(bf))            # [D, P, K1, H1]
    w2t = np.ascontiguousarray(
        W2.transpose(0, 2, 1).reshape(D, K2, P, H2)
        .transpose(0, 2, 1, 3).astype(bf))            # [D, P, K2, H2]
    b3p = np.ascontiguousarray(b3.reshape(1, D))

    ids = feat_ids.astype(np.int64)
    in_maps = []
    for c in range(NCORES):
        idc = ids[c * BC:(c + 1) * BC]                # [BC, F]
        g = idc.reshape(NBT, P, F).transpose(1, 0, 2)
        g = g + (np.arange(F, dtype=np.int64) * V)[None, None, :]
        gidx = np.ascontiguousarray(g.reshape(P, NBT * F).astype(np.int32))
        in_maps.append({
            "tab": tab, "gidx": gidx,
            "w1t": w1t, "w2t": w2t,
            "g1": g1, "be1": be1, "g2": g2, "be2": be2,
            "w3": W3.astype(bf), "b3p": b3p,
        })

    res = run_bass_kernel_spmd(
        _NC, in_maps, core_ids=list(range(NCORES)), trace=bool(PROFILE))
    if PROFILE:
        LAST_EXEC_NS = res.exec_time_ns

    out_full = np.concatenate(
        [res.results[c]["out"] for c in range(NCORES)], axis=1)  # [D, B]
    final = out_full[domain_id.astype(np.int64), np.arange(B)]
    return final.astype(np.float32)


# revision 4
# speedup vs baseline: 1.1370x; 1.1080x over previous
"""Trainium2 Bass kernel for nn_Mlp_2_Layer (moe_routing) — v3.

Data-parallel over batch (1024 samples/core, all 8 domains/core), with
the three big baseline losses fixed:

1. No L1 recompute: the L1 pre-activations are copied PSUM->SBUF bf16
   (128KB/partition tile) and re-read for the ReLU; the baseline ran
   the 512 L1 matmuls twice.
2. Less collective stall: the BN-stats AllReduces are split into
   domain halves and kicked as soon as each half's partials are
   ready, so most of their latency hides under remaining compute.
   (A collective blocks the Pool dynamic-DMA ring until it completes,
   so they can only sit after all the gathers in the Pool queue.)
3. No PE time on transposes / gather-chasing head: the embedding table
   is bf16, gathered per (feature, batch-tile) with indirect DMA
   (Pool), transposed feature-major by the DMA xbar
   (dma_start_transpose), and L1 runs in bf16 at N=256 granularity so
   the tensor engine chases the gather stream; bf16 matmuls run at
   full PE rate (1 row/cycle) like f32r but halve SBUF/DMA.

h2 (the L2 pre-activations, needed again after the BN2 stats
AllReduce) is written into the SBUF space of pre1 that the same
(domain, chunk) just consumed — no HBM spill.

Final pass (per domain): a2 = relu(s2*h2+t2), W3-column matmuls reduce
over the 512 h2 partitions, sigmoid(+b3). P3 for domains 0-3 is
interleaved with L2 of domains 4-7 to hide the second stats AllReduce.

Host combines: final[b] = out[domain_id[b], b].
"""
import sys

for _p in ("/opt/trn_rl_repo", "/root/.axon_site"):
    if _p not in sys.path:
        sys.path.insert(0, _p)

import numpy as np

B, F, E, V = 8192, 16, 32, 100000
D, H1, H2 = 8, 1024, 512
IN = F * E          # 512
EPS = 1e-5
NCORES = 8
BC = B // NCORES    # 1024 samples per core
P = 128
NBT = BC // P       # 8 batch tiles per core
NBP = NBT // 2      # 4 batch-tile pairs (N=256 L1 chunks)
NT = BC // 512      # 2 n-chunks of 512 per core
K1 = IN // P        # 4
M1 = H1 // P        # 8
K2 = H1 // P        # 8
M2 = H2 // P        # 4
HD = D // 2         # stats collectives split into two domain halves

PROFILE = False
LAST_EXEC_NS = None

_NC = None


def _build():
    import concourse.bass as bass
    import concourse.tile as tile
    from concourse import bacc, mybir
    from concourse.masks import make_identity
    from contextlib import ExitStack

    f32 = mybir.dt.float32
    bf16 = mybir.dt.bfloat16
    i32 = mybir.dt.int32
    AF = mybir.ActivationFunctionType
    ALU = mybir.AluOpType

    nc = bacc.Bacc(None, target_bir_lowering=False, debug=False)

    tab_d = nc.dram_tensor("tab", [F * V, E], bf16, kind="ExternalInput")
    gidx_d = nc.dram_tensor("gidx", [P, NBT * F], i32, kind="ExternalInput")
    w1t_d = nc.dram_tensor("w1t", [D, P, K1, H1], bf16, kind="ExternalInput")
    w2t_d = nc.dram_tensor("w2t", [D, P, K2, H2], bf16, kind="ExternalInput")
    g1_d = nc.dram_tensor("g1", [D, H1], f32, kind="ExternalInput")
    be1_d = nc.dram_tensor("be1", [D, H1], f32, kind="ExternalInput")
    g2_d = nc.dram_tensor("g2", [D, H2], f32, kind="ExternalInput")
    be2_d = nc.dram_tensor("be2", [D, H2], f32, kind="ExternalInput")
    w3_d = nc.dram_tensor("w3", [D, H2], bf16, kind="ExternalInput")
    b3_d = nc.dram_tensor("b3p", [1, D], f32, kind="ExternalInput")
    out_d = nc.dram_tensor("out", [D, BC], f32, kind="ExternalOutput")

    # collectives: 2 halves x 2 layers (NO warmup AR: a collective
    # blocks the Pool qPoolDynamic ring until it completes, so a warmup
    # at t=0 just stalls the gathers that feed everything)
    n1 = 2 * HD * M1
    n2 = 2 * HD * M2
    cc_sizes = [n1, n1, n2, n2, 8]
    cc_in = [nc.dram_tensor(f"cci{i}", [P, s], f32, kind="Internal")
             for i, s in enumerate(cc_sizes)]
    cc_out = [nc.dram_tensor(f"cco{i}", [P, s], f32, kind="Internal",
                             addr_space="Shared")
              for i, s in enumerate(cc_sizes)]
    RG = [list(range(NCORES))]

    with tile.TileContext(nc) as tc:
        with ExitStack() as ctx:
            const = ctx.enter_context(tc.tile_pool(name="const", bufs=1))
            stp = ctx.enter_context(tc.tile_pool(name="stp", bufs=1))
            gpool = ctx.enter_context(tc.tile_pool(name="gpool", bufs=4))
            wpool = ctx.enter_context(tc.tile_pool(name="wpool", bufs=2))
            a1p = ctx.enter_context(tc.tile_pool(name="a1p", bufs=2))
            a2p = ctx.enter_context(tc.tile_pool(name="a2p", bufs=2))
            sgp = ctx.enter_context(tc.tile_pool(name="sgp", bufs=2))
            l1ps = ctx.enter_context(tc.tile_pool(name="l1ps", bufs=3, space="PSUM"))
            l2ps = ctx.enter_context(tc.tile_pool(name="l2ps", bufs=3, space="PSUM"))
            w3ps = ctx.enter_context(tc.tile_pool(name="w3ps", bufs=1, space="PSUM"))
            tpps = ctx.enter_context(tc.tile_pool(name="tpps", bufs=1, space="PSUM"))

            # ---- consts ----
            eps_t = const.tile([P, 1], f32)
            nc.vector.memset(eps_t[:], EPS)
            ident = const.tile([P, P], bf16)
            make_identity(nc, ident[:])

            gidx = const.tile([P, NBT * F], i32)
            nc.sync.dma_start(out=gidx[:], in_=gidx_d[:, :])
            g1c = const.tile([P, D * M1], f32)
            nc.sync.dma_start(out=g1c[:], in_=g1_d[:, :].rearrange(
                "d (m p) -> p (d m)", p=P))
            be1c = const.tile([P, D * M1], f32)
            nc.sync.dma_start(out=be1c[:], in_=be1_d[:, :].rearrange(
                "d (m p) -> p (d m)", p=P))
            g2c = const.tile([P, D * M2], f32)
            nc.sync.dma_start(out=g2c[:], in_=g2_d[:, :].rearrange(
                "d (m p) -> p (d m)", p=P))
            be2c = const.tile([P, D * M2], f32)
            nc.sync.dma_start(out=be2c[:], in_=be2_d[:, :].rearrange(
                "d (m p) -> p (d m)", p=P))
            w3c = const.tile([P, D * M2], bf16)
            nc.sync.dma_start(out=w3c[:], in_=w3_d[:, :].rearrange(
                "d (m p) -> p (d m)", p=P))
            b3p = const.tile([1, D], f32)
            nc.sync.dma_start(out=b3p[:], in_=b3_d[:, :])

            # ---- persistent state ----
            xt = stp.tile([P, K1, BC], bf16, name="xt")
            pre1 = stp.tile([P, D, M1, BC], bf16, name="pre1")
            st1 = [stp.tile([P, HD, M1, NBP, 6], f32, name=f"st1_{h}")
                   for h in range(2)]
            mv1 = [stp.tile([P, HD, M1, 2], f32, name=f"mv1_{h}")
                   for h in range(2)]
            st2 = [stp.tile([P, HD, M2, NT, 6], f32, name=f"st2_{h}")
                   for h in range(2)]
            mv2 = [stp.tile([P, HD, M2, 2], f32, name=f"mv2_{h}")
                   for h in range(2)]
            s1 = [stp.tile([P, HD * M1], f32, name=f"s1_{h}") for h in range(2)]
            t1 = [stp.tile([P, HD * M1], f32, name=f"t1_{h}") for h in range(2)]
            s2 = [stp.tile([P, HD * M2], f32, name=f"s2_{h}") for h in range(2)]
            t2 = [stp.tile([P, HD * M2], f32, name=f"t2_{h}") for h in range(2)]
            uq1 = [stp.tile([P, n1], f32, name=f"uq1_{h}") for h in range(2)]
            sa1 = [stp.tile([P, n1], f32, name=f"sa1_{h}") for h in range(2)]
            uq2 = [stp.tile([P, n2], f32, name=f"uq2_{h}") for h in range(2)]
            sa2 = [stp.tile([P, n2], f32, name=f"sa2_{h}") for h in range(2)]
            tmps = [stp.tile([P, HD * M1], f32, name=f"tmp_{i}")
                    for i in range(4)]

            def stats_pack(h, M, st, mv, uq, cci, cco):
                """Per-core (mean, E[x^2]) for domain half h -> AllReduce."""
                for dd in range(HD):
                    for m in range(M):
                        nc.vector.bn_aggr(out=mv[:, dd, m, :],
                                          in_=st[:, dd, m, :, :])
                n = HD * M
                u = uq[:, 0:n].rearrange("p (d m) -> p d m", d=HD)
                q = uq[:, n:].rearrange("p (d m) -> p d m", d=HD)
                nc.vector.tensor_copy(out=u, in_=mv[:, :, :, 0])
                nc.vector.tensor_mul(out=q, in0=mv[:, :, :, 0],
                                     in1=mv[:, :, :, 0])
                nc.vector.tensor_add(out=q, in0=q, in1=mv[:, :, :, 1])
                nc.gpsimd.dma_start(out=cci[:, :], in_=uq[:])
                nc.gpsimd.collective_compute(
                    "AllReduce", ALU.add, replica_groups=RG,
                    ins=[cci[:, :]], outs=[cco[:, :]])

            def stats_apply(idx, h, M, sa, cco, g_c, be_c, s_t, t_t):
                nc.gpsimd.dma_start(out=sa[:], in_=cco[:, :])
                n = HD * M
                lo = h * HD
                mean = tmps[idx][:, 0:n]
                var = sa[:, n:]
                nc.vector.tensor_scalar_mul(mean, sa[:, 0:n], 1.0 / NCORES)
                nc.vector.tensor_scalar_mul(var, var, 1.0 / NCORES)
                gl = slice(lo * M, (lo + HD) * M)
                nc.vector.tensor_mul(out=s_t[:], in0=mean, in1=mean)
                nc.vector.tensor_tensor(out=var, in0=var, in1=s_t[:],
                                        op=ALU.subtract)
                nc.scalar.activation(out=var, in_=var, func=AF.Sqrt,
                                     bias=eps_t[:], scale=1.0)
                nc.vector.reciprocal(out=var, in_=var)
                nc.vector.tensor_mul(out=s_t[:], in0=g_c[:, gl], in1=var)
                nc.vector.tensor_mul(out=t_t[:], in0=mean, in1=s_t[:])
                nc.vector.tensor_tensor(out=t_t[:], in0=be_c[:, gl],
                                        in1=t_t[:], op=ALU.subtract)

            # ---- P1: gather + xbar-transpose + L1 (N=256, chases gather) ----
            for bp in range(NBP):
                for bt2 in range(2):
                    bt = 2 * bp + bt2
                    G = gpool.tile([P, F, E], bf16, tag="G")
                    for f in range(F):
                        nc.gpsimd.indirect_dma_start(
                            out=G[:, f, :],
                            out_offset=None,
                            in_=tab_d[:, :],
                            in_offset=bass.IndirectOffsetOnAxis(
                                ap=gidx[:, bt * F + f: bt * F + f + 1],
                                axis=0),
                        )
                    gf = G[:].rearrange("p f e -> p (f e)")
                    tp = tpps.tile([P, K1 * P], bf16, tag="tp")
                    for k in range(K1):
                        nc.tensor.transpose(
                            out=tp[:, k * P:(k + 1) * P],
                            in_=gf[:, k * P:(k + 1) * P],
                            identity=ident[:])
                    nc.vector.tensor_copy(
                        out=xt[:, :, bt * P:(bt + 1) * P],
                        in_=tp[:].rearrange("p (k c) -> p k c", k=K1))
                if bp == NBP - 1:
                    # CC warmup right after the last gather: Pool's dynamic
                    # ring is idle from here on, and the first NRT collective
                    # of an execution costs ~40-90us — pay it before the real
                    # stats AllReduces need the CC path.
                    nc.gpsimd.dma_start(out=cc_in[4][:, :], in_=g1c[:, 0:8])
                    nc.gpsimd.collective_compute(
                        "AllReduce", ALU.add, replica_groups=RG,
                        ins=[cc_in[4][:, :]], outs=[cc_out[4][:, :]])
                for d in range(D):
                    w1 = wpool.tile([P, K1, H1], bf16, tag="w")
                    nc.sync.dma_start(out=w1[:], in_=w1t_d[d, :, :, :])
                    csl = slice(bp * 256, (bp + 1) * 256)
                    for m in range(M1):
                        pm = l1ps.tile([P, 256], f32, tag="pm")
                        for k in range(K1):
                            nc.tensor.matmul(
                                out=pm[:],
                                lhsT=w1[:, k, m * P:(m + 1) * P],
                                rhs=xt[:, k, csl],
                                start=(k == 0), stop=(k == K1 - 1))
                        nc.vector.bn_stats(
                            out=st1[d // HD][:, d % HD, m, bp, :], in_=pm[:])
                        nc.scalar.copy(out=pre1[:, d, m, csl], in_=pm[:])
                    if bp == NBP - 1 and d == HD - 1:
                        stats_pack(0, M1, st1[0], mv1[0], uq1[0],
                                   cc_in[0], cc_out[0])
            stats_pack(1, M1, st1[1], mv1[1], uq1[1], cc_in[1], cc_out[1])
            stats_apply(0, 0, M1, sa1[0], cc_out[0], g1c, be1c, s1[0], t1[0])

            # ---- P2: a1 = relu(s1*pre1+t1); L2; h2 into pre1's dead space --
            def p3_domain(d):
                hh = d // HD
                a2 = a2p.tile([P, M2, BC], bf16, tag="a2")
                for m2 in range(M2):
                    dm = (d % HD) * M2 + m2
                    nc.scalar.activation(
                        out=a2[:, m2, :], in_=pre1[:, d, m2, :],
                        func=AF.Relu,
                        bias=t2[hh][:, dm:dm + 1],
                        scale=s2[hh][:, dm:dm + 1])
                for nt in range(NT):
                    nsl = slice(nt * 512, (nt + 1) * 512)
                    po = w3ps.tile([1, 512], f32, tag="po")
                    for m2 in range(M2):
                        nc.tensor.matmul(
                            out=po[:],
                            lhsT=w3c[:, d * M2 + m2:d * M2 + m2 + 1],
                            rhs=a2[:, m2, nsl],
                            start=(m2 == 0), stop=(m2 == M2 - 1))
                    sg = sgp.tile([1, 512], f32, tag="sg")
                    nc.scalar.activation(out=sg[:], in_=po[:], func=AF.Sigmoid,
                                         bias=b3p[0:1, d:d + 1], scale=1.0)
                    nc.sync.dma_start(out=out_d[d, nsl], in_=sg[:])

            for d in range(D):
                hh = d // HD
                if d == HD:
                    stats_apply(1, 1, M1, sa1[1], cc_out[1], g1c, be1c,
                                s1[1], t1[1])
                w2 = wpool.tile([P, K2, H2], bf16, tag="w")
                nc.sync.dma_start(out=w2[:], in_=w2t_d[d, :, :, :])
                for nt in range(NT):
                    nsl = slice(nt * 512, (nt + 1) * 512)
                    a1 = a1p.tile([P, K2, 512], bf16, tag="a1")
                    for m in range(M1):
                        dm = (d % HD) * M1 + m
                        nc.scalar.activation(
                            out=a1[:, m, :], in_=pre1[:, d, m, nsl],
                            func=AF.Relu,
                            bias=t1[hh][:, dm:dm + 1],
                            scale=s1[hh][:, dm:dm + 1])
                    for m2 in range(M2):
                        pm2 = l2ps.tile([P, 512], f32, tag="pm2")
                        for k2 in range(K2):
                            nc.tensor.matmul(
                                out=pm2[:],
                                lhsT=w2[:, k2, m2 * P:(m2 + 1) * P],
                                rhs=a1[:, k2, :],
                                start=(k2 == 0), stop=(k2 == K2 - 1))
                        nc.vector.bn_stats(
                            out=st2[d // HD][:, d % HD, m2, nt, :],
                            in_=pm2[:])
                        # h2 reuses the pre1 slot this (d, nt) just consumed
                        nc.vector.tensor_copy(out=pre1[:, d, m2, nsl],
                                              in_=pm2[:])
                if d == HD - 1:
                    stats_pack(0, M2, st2[0], mv2[0], uq2[0],
                               cc_in[2], cc_out[2])
                if d == HD:
                    stats_apply(2, 0, M2, sa2[0], cc_out[2], g2c, be2c,
                                s2[0], t2[0])
                if d >= HD:
                    # interleave P3 of domain (d-HD) under L2 of domain d
                    p3_domain(d - HD)
            stats_pack(1, M2, st2[1], mv2[1], uq2[1], cc_in[3], cc_out[3])
            stats_apply(3, 1, M2, sa2[1], cc_out[3], g2c, be2c, s2[1], t2[1])
            for d in range(HD, D):
                p3_domain(d)

    nc.compile()
    return nc


def kernel(**inputs):
    global _NC, LAST_EXEC_NS
    from concourse.bass_utils import run_bass_kernel_spmd
    import ml_dtypes

    bf = ml_dtypes.bfloat16

    feat_ids = np.asarray(inputs["feat_ids"])
    domain_id = np.asarray(inputs["domain_id"])
    emb_tables = np.asarray(inputs["emb_tables"], dtype=np.float32)
    W1 = np.asarray(inputs["W1"], dtype=np.float32)
    g1 = np.asarray(inputs["g1"], dtype=np.float32)
    be1 = np.asarray(inputs["be1"], dtype=np.float32)
    W2 = np.asarray(inputs["W2"], dtype=np.float32)
    g2 = np.asarray(inputs["g2"], dtype=np.float32)
    be2 = np.asarray(inputs["be2"], dtype=np.float32)
    W3 = np.asarray(inputs["W3"], dtype=np.float32)
    b3 = np.asarray(inputs["b3"], dtype=np.float32)
    # b1/b2 cancel inside training-mode BatchNorm (constant shift).

    if _NC is None:
        _NC = _build()

    tab = np.ascontiguousarray(emb_tables.reshape(F * V, E).astype(bf))
    w1t = np.ascontiguousarray(
        W1.transpose(0, 2, 1).reshape(D, K1, P, H1)
        .transpose(0, 2, 1, 3).astype(bf))            # [D, P, K1, H1]
    w2t = np.ascontiguousarray(
        W2.transpose(0, 2, 1).reshape(D, K2, P, H2)
        .transpose(0, 2, 1, 3).astype(bf))            # [D, P, K2, H2]
    b3p = np.ascontiguousarray(b3.reshape(1, D))

    ids = feat_ids.astype(np.int64)
    in_maps = []
    for c in range(NCORES):
        idc = ids[c * BC:(c + 1) * BC]                # [BC, F]
        g = idc.reshape(NBT, P, F).transpose(1, 0, 2)
        g = g + (np.arange(F, dtype=np.int64) * V)[None, None, :]
        gidx = np.ascontiguousarray(g.reshape(P, NBT * F).astype(np.int32))
        in_maps.append({
            "tab": tab, "gidx": gidx,
            "w1t": w1t, "w2t": w2t,
            "g1": g1, "be1": be1, "g2": g2, "be2": be2,
            "w3": W3.astype(bf), "b3p": b3p,
        })

    res = run_bass_kernel_spmd(
        _NC, in_maps, core_ids=list(range(NCORES)), trace=bool(PROFILE))
    if PROFILE:
        LAST_EXEC_NS = res.exec_time_ns

    out_full = np.concatenate(
        [res.results[c]["out"] for c in range(NCORES)], axis=1)  # [D, B]
    final = out_full[domain_id.astype(np.int64), np.arange(B)]
    return final.astype(np.float32)


# revision 5
# speedup vs baseline: 1.1418x; 1.0042x over previous
"""Trainium2 Bass kernel for nn_Mlp_2_Layer (moe_routing) — v3.

Data-parallel over batch (1024 samples/core, all 8 domains/core), with
the three big baseline losses fixed:

1. No L1 recompute: the L1 pre-activations are copied PSUM->SBUF bf16
   (128KB/partition tile) and re-read for the ReLU; the baseline ran
   the 512 L1 matmuls twice.
2. Less collective stall: the BN-stats AllReduces are split into
   domain halves and kicked as soon as each half's partials are
   ready, so most of their latency hides under remaining compute.
   (A collective blocks the Pool dynamic-DMA ring until it completes,
   so they can only sit after all the gathers in the Pool queue.)
3. No PE time on transposes / gather-chasing head: the embedding table
   is bf16, gathered per (feature, batch-tile) with indirect DMA
   (Pool), transposed feature-major by the DMA xbar
   (dma_start_transpose), and L1 runs in bf16 at N=256 granularity so
   the tensor engine chases the gather stream; bf16 matmuls run at
   full PE rate (1 row/cycle) like f32r but halve SBUF/DMA.

h2 (the L2 pre-activations, needed again after the BN2 stats
AllReduce) is written into the SBUF space of pre1 that the same
(domain, chunk) just consumed — no HBM spill.

Final pass (per domain): a2 = relu(s2*h2+t2), W3-column matmuls reduce
over the 512 h2 partitions, sigmoid(+b3). P3 for domains 0-3 is
interleaved with L2 of domains 4-7 to hide the second stats AllReduce.

Host combines: final[b] = out[domain_id[b], b].
"""
import sys

for _p in ("/opt/trn_rl_repo", "/root/.axon_site"):
    if _p not in sys.path:
        sys.path.insert(0, _p)

import numpy as np

B, F, E, V = 8192, 16, 32, 100000
D, H1, H2 = 8, 1024, 512
IN = F * E          # 512
EPS = 1e-5
NCORES = 8
BC = B // NCORES    # 1024 samples per core
P = 128
NBT = BC // P       # 8 batch tiles per core
NBP = NBT // 2      # 4 batch-tile pairs (N=256 L1 chunks)
NT = BC // 512      # 2 n-chunks of 512 per core
K1 = IN // P        # 4
M1 = H1 // P        # 8
K2 = H1 // P        # 8
M2 = H2 // P        # 4
HD = D // 2         # stats collectives split into two domain halves

PROFILE = False
LAST_EXEC_NS = None

_NC = None


def _build():
    import concourse.bass as bass
    import concourse.tile as tile
    from concourse import bacc, mybir
    from concourse.masks import make_identity
    from contextlib import ExitStack

    f32 = mybir.dt.float32
    bf16 = mybir.dt.bfloat16
    i32 = mybir.dt.int32
    AF = mybir.ActivationFunctionType
    ALU = mybir.AluOpType

    nc = bacc.Bacc(None, target_bir_lowering=False, debug=False)

    tab_d = nc.dram_tensor("tab", [F * V, E], bf16, kind="ExternalInput")
    gidx_d = nc.dram_tensor("gidx", [P, NBT * F], i32, kind="ExternalInput")
    w1t_d = nc.dram_tensor("w1t", [D, P, K1, H1], bf16, kind="ExternalInput")
    w2t_d = nc.dram_tensor("w2t", [D, P, K2, H2], bf16, kind="ExternalInput")
    g1_d = nc.dram_tensor("g1", [D, H1], f32, kind="ExternalInput")
    be1_d = nc.dram_tensor("be1", [D, H1], f32, kind="ExternalInput")
    g2_d = nc.dram_tensor("g2", [D, H2], f32, kind="ExternalInput")
    be2_d = nc.dram_tensor("be2", [D, H2], f32, kind="ExternalInput")
    w3_d = nc.dram_tensor("w3", [D, H2], bf16, kind="ExternalInput")
    b3_d = nc.dram_tensor("b3p", [1, D], f32, kind="ExternalInput")
    out_d = nc.dram_tensor("out", [D, BC], f32, kind="ExternalOutput")

    # collectives: 2 halves x 2 layers (NO warmup AR: a collective
    # blocks the Pool qPoolDynamic ring until it completes, so a warmup
    # at t=0 just stalls the gathers that feed everything)
    n1 = 2 * HD * M1
    n2 = 2 * HD * M2
    cc_sizes = [n1, n1, n2, n2, 8]
    cc_in = [nc.dram_tensor(f"cci{i}", [P, s], f32, kind="Internal")
             for i, s in enumerate(cc_sizes)]
    cc_out = [nc.dram_tensor(f"cco{i}", [P, s], f32, kind="Internal",
                             addr_space="Shared")
              for i, s in enumerate(cc_sizes)]
    RG = [list(range(NCORES))]

    with tile.TileContext(nc) as tc:
        with ExitStack() as ctx:
            const = ctx.enter_context(tc.tile_pool(name="const", bufs=1))
            stp = ctx.enter_context(tc.tile_pool(name="stp", bufs=1))
            gpool = ctx.enter_context(tc.tile_pool(name="gpool", bufs=4))
            wpool = ctx.enter_context(tc.tile_pool(name="wpool", bufs=2))
            a1p = ctx.enter_context(tc.tile_pool(name="a1p", bufs=2))
            a2p = ctx.enter_context(tc.tile_pool(name="a2p", bufs=2))
            sgp = ctx.enter_context(tc.tile_pool(name="sgp", bufs=2))
            l1ps = ctx.enter_context(tc.tile_pool(name="l1ps", bufs=3, space="PSUM"))
            l2ps = ctx.enter_context(tc.tile_pool(name="l2ps", bufs=3, space="PSUM"))
            w3ps = ctx.enter_context(tc.tile_pool(name="w3ps", bufs=1, space="PSUM"))
            tpps = ctx.enter_context(tc.tile_pool(name="tpps", bufs=1, space="PSUM"))

            # ---- consts ----
            eps_t = const.tile([P, 1], f32)
            nc.vector.memset(eps_t[:], EPS)
            ident = const.tile([P, P], bf16)
            make_identity(nc, ident[:])

            gidx = const.tile([P, NBT * F], i32)
            nc.sync.dma_start(out=gidx[:], in_=gidx_d[:, :])
            g1c = const.tile([P, D * M1], f32)
            nc.sync.dma_start(out=g1c[:], in_=g1_d[:, :].rearrange(
                "d (m p) -> p (d m)", p=P))
            be1c = const.tile([P, D * M1], f32)
            nc.sync.dma_start(out=be1c[:], in_=be1_d[:, :].rearrange(
                "d (m p) -> p (d m)", p=P))
            g2c = const.tile([P, D * M2], f32)
            nc.sync.dma_start(out=g2c[:], in_=g2_d[:, :].rearrange(
                "d (m p) -> p (d m)", p=P))
            be2c = const.tile([P, D * M2], f32)
            nc.sync.dma_start(out=be2c[:], in_=be2_d[:, :].rearrange(
                "d (m p) -> p (d m)", p=P))
            w3c = const.tile([P, D * M2], bf16)
            nc.sync.dma_start(out=w3c[:], in_=w3_d[:, :].rearrange(
                "d (m p) -> p (d m)", p=P))
            b3p = const.tile([1, D], f32)
            nc.sync.dma_start(out=b3p[:], in_=b3_d[:, :])

            # ---- persistent state ----
            xt = stp.tile([P, K1, BC], bf16, name="xt")
            pre1 = stp.tile([P, D, M1, BC], bf16, name="pre1")
            st1 = [stp.tile([P, HD, M1, NT, 6], f32, name=f"st1_{h}")
                   for h in range(2)]
            mv1 = [stp.tile([P, HD, M1, 2], f32, name=f"mv1_{h}")
                   for h in range(2)]
            st2 = [stp.tile([P, HD, M2, NT, 6], f32, name=f"st2_{h}")
                   for h in range(2)]
            mv2 = [stp.tile([P, HD, M2, 2], f32, name=f"mv2_{h}")
                   for h in range(2)]
            s1 = [stp.tile([P, HD * M1], f32, name=f"s1_{h}") for h in range(2)]
            t1 = [stp.tile([P, HD * M1], f32, name=f"t1_{h}") for h in range(2)]
            s2 = [stp.tile([P, HD * M2], f32, name=f"s2_{h}") for h in range(2)]
            t2 = [stp.tile([P, HD * M2], f32, name=f"t2_{h}") for h in range(2)]
            uq1 = [stp.tile([P, n1], f32, name=f"uq1_{h}") for h in range(2)]
            sa1 = [stp.tile([P, n1], f32, name=f"sa1_{h}") for h in range(2)]
            uq2 = [stp.tile([P, n2], f32, name=f"uq2_{h}") for h in range(2)]
            sa2 = [stp.tile([P, n2], f32, name=f"sa2_{h}") for h in range(2)]
            tmps = [stp.tile([P, HD * M1], f32, name=f"tmp_{i}")
                    for i in range(4)]

            def stats_pack(h, M, st, mv, uq, cci, cco):
                """Per-core (mean, E[x^2]) for domain half h -> AllReduce."""
                for dd in range(HD):
                    for m in range(M):
                        nc.vector.bn_aggr(out=mv[:, dd, m, :],
                                          in_=st[:, dd, m, :, :])
                n = HD * M
                u = uq[:, 0:n].rearrange("p (d m) -> p d m", d=HD)
                q = uq[:, n:].rearrange("p (d m) -> p d m", d=HD)
                nc.vector.tensor_copy(out=u, in_=mv[:, :, :, 0])
                nc.vector.tensor_mul(out=q, in0=mv[:, :, :, 0],
                                     in1=mv[:, :, :, 0])
                nc.vector.tensor_add(out=q, in0=q, in1=mv[:, :, :, 1])
                nc.gpsimd.dma_start(out=cci[:, :], in_=uq[:])
                nc.gpsimd.collective_compute(
                    "AllReduce", ALU.add, replica_groups=RG,
                    ins=[cci[:, :]], outs=[cco[:, :]])

            def stats_apply(idx, h, M, sa, cco, g_c, be_c, s_t, t_t):
                nc.gpsimd.dma_start(out=sa[:], in_=cco[:, :])
                n = HD * M
                lo = h * HD
                mean = tmps[idx][:, 0:n]
                var = sa[:, n:]
                nc.vector.tensor_scalar_mul(mean, sa[:, 0:n], 1.0 / NCORES)
                nc.vector.tensor_scalar_mul(var, var, 1.0 / NCORES)
                gl = slice(lo * M, (lo + HD) * M)
                nc.vector.tensor_mul(out=s_t[:], in0=mean, in1=mean)
                nc.vector.tensor_tensor(out=var, in0=var, in1=s_t[:],
                                        op=ALU.subtract)
                nc.scalar.activation(out=var, in_=var, func=AF.Sqrt,
                                     bias=eps_t[:], scale=1.0)
                nc.vector.reciprocal(out=var, in_=var)
                nc.vector.tensor_mul(out=s_t[:], in0=g_c[:, gl], in1=var)
                nc.vector.tensor_mul(out=t_t[:], in0=mean, in1=s_t[:])
                nc.vector.tensor_tensor(out=t_t[:], in0=be_c[:, gl],
                                        in1=t_t[:], op=ALU.subtract)

            # ---- P1: gather + PE-transpose + L1 (N=512: a [128,128,512]
            # matmul runs ~215ns (LDWEIGHTS fully hidden) vs ~390ns at N=256,
            # which outweighs the lost gather-chase overlap) ----
            for nt in range(NT):
                for bt2 in range(4):
                    bt = 4 * nt + bt2
                    G = gpool.tile([P, F, E], bf16, tag="G")
                    for f in range(F):
                        nc.gpsimd.indirect_dma_start(
                            out=G[:, f, :],
                            out_offset=None,
                            in_=tab_d[:, :],
                            in_offset=bass.IndirectOffsetOnAxis(
                                ap=gidx[:, bt * F + f: bt * F + f + 1],
                                axis=0),
                        )
                    gf = G[:].rearrange("p f e -> p (f e)")
                    tp = tpps.tile([P, K1 * P], bf16, tag="tp")
                    for k in range(K1):
                        nc.tensor.transpose(
                            out=tp[:, k * P:(k + 1) * P],
                            in_=gf[:, k * P:(k + 1) * P],
                            identity=ident[:])
                    nc.vector.tensor_copy(
                        out=xt[:, :, bt * P:(bt + 1) * P],
                        in_=tp[:].rearrange("p (k c) -> p k c", k=K1))
                if nt == NT - 1:
                    # CC warmup right after the last gather: Pool's dynamic
                    # ring is idle from here on, and the first NRT collective
                    # of an execution costs ~40-90us — pay it before the real
                    # stats AllReduces need the CC path.
                    nc.gpsimd.dma_start(out=cc_in[4][:, :], in_=g1c[:, 0:8])
                    nc.gpsimd.collective_compute(
                        "AllReduce", ALU.add, replica_groups=RG,
                        ins=[cc_in[4][:, :]], outs=[cc_out[4][:, :]])
                for d in range(D):
                    w1 = wpool.tile([P, K1, H1], bf16, tag="w")
                    nc.sync.dma_start(out=w1[:], in_=w1t_d[d, :, :, :])
                    csl = slice(nt * 512, (nt + 1) * 512)
                    for m in range(M1):
                        pm = l1ps.tile([P, 512], f32, tag="pm")
                        for k in range(K1):
                            nc.tensor.matmul(
                                out=pm[:],
                                lhsT=w1[:, k, m * P:(m + 1) * P],
                                rhs=xt[:, k, csl],
                                start=(k == 0), stop=(k == K1 - 1))
                        nc.vector.bn_stats(
                            out=st1[d // HD][:, d % HD, m, nt, :], in_=pm[:])
                        nc.scalar.copy(out=pre1[:, d, m, csl], in_=pm[:])
                    if nt == NT - 1 and d == HD - 1:
                        stats_pack(0, M1, st1[0], mv1[0], uq1[0],
                                   cc_in[0], cc_out[0])
            stats_pack(1, M1, st1[1], mv1[1], uq1[1], cc_in[1], cc_out[1])
            stats_apply(0, 0, M1, sa1[0], cc_out[0], g1c, be1c, s1[0], t1[0])

            # ---- P2: a1 = relu(s1*pre1+t1); L2; h2 into pre1's dead space --
            def p3_domain(d):
                hh = d // HD
                a2 = a2p.tile([P, M2, BC], bf16, tag="a2")
                for m2 in range(M2):
                    dm = (d % HD) * M2 + m2
                    nc.scalar.activation(
                        out=a2[:, m2, :], in_=pre1[:, d, m2, :],
                        func=AF.Relu,
                        bias=t2[hh][:, dm:dm + 1],
                        scale=s2[hh][:, dm:dm + 1])
                for nt in range(NT):
                    nsl = slice(nt * 512, (nt + 1) * 512)
                    po = w3ps.tile([1, 512], f32, tag="po")
                    for m2 in range(M2):
                        nc.tensor.matmul(
                            out=po[:],
                            lhsT=w3c[:, d * M2 + m2:d * M2 + m2 + 1],
                            rhs=a2[:, m2, nsl],
                            start=(m2 == 0), stop=(m2 == M2 - 1))
                    sg = sgp.tile([1, 512], f32, tag="sg")
                    nc.scalar.activation(out=sg[:], in_=po[:], func=AF.Sigmoid,
                                         bias=b3p[0:1, d:d + 1], scale=1.0)
                    nc.sync.dma_start(out=out_d[d, nsl], in_=sg[:])

            for d in range(D):
                hh = d // HD
                if d == HD:
                    stats_apply(1, 1, M1, sa1[1], cc_out[1], g1c, be1c,
                                s1[1], t1[1])
                w2 = wpool.tile([P, K2, H2], bf16, tag="w")
                nc.sync.dma_start(out=w2[:], in_=w2t_d[d, :, :, :])
                for nt in range(NT):
                    nsl = slice(nt * 512, (nt + 1) * 512)
                    a1 = a1p.tile([P, K2, 512], bf16, tag="a1")
                    for m in range(M1):
                        dm = (d % HD) * M1 + m
                        nc.scalar.activation(
                            out=a1[:, m, :], in_=pre1[:, d, m, nsl],
                            func=AF.Relu,
                            bias=t1[hh][:, dm:dm + 1],
                            scale=s1[hh][:, dm:dm + 1])
                    for m2 in range(M2):
                        pm2 = l2ps.tile([P, 512], f32, tag="pm2")
                        for k2 in range(K2):
                            nc.tensor.matmul(
                                out=pm2[:],
                                lhsT=w2[:, k2, m2 * P:(m2 + 1) * P],
                                rhs=a1[:, k2, :],
                                start=(k2 == 0), stop=(k2 == K2 - 1))
                        nc.vector.bn_stats(
                            out=st2[d // HD][:, d % HD, m2, nt, :],
                            in_=pm2[:])
                        # h2 reuses the pre1 slot this (d, nt) just consumed
                        nc.vector.tensor_copy(out=pre1[:, d, m2, nsl],
                                              in_=pm2[:])
                if d == HD - 1:
                    stats_pack(0, M2, st2[0], mv2[0], uq2[0],
                               cc_in[2], cc_out[2])
                if d == HD:
                    stats_apply(2, 0, M2, sa2[0], cc_out[2], g2c, be2c,
                                s2[0], t2[0])
                if d >= HD:
                    # interleave P3 of domain (d-HD) under L2 of domain d
                    p3_domain(d - HD)
            stats_pack(1, M2, st2[1], mv2[1], uq2[1], cc_in[3], cc_out[3])
            stats_apply(3, 1, M2, sa2[1], cc_out[3], g2c, be2c, s2[1], t2[1])
            for d in range(HD, D):
                p3_domain(d)

    nc.compile()
    return nc


def kernel(**inputs):
    global _NC, LAST_EXEC_NS
    from concourse.bass_utils import run_bass_kernel_spmd
    import ml_dtypes

    bf = ml_dtypes.bfloat16

    feat_ids = np.asarray(inputs["feat_ids"])
    domain_id = np.asarray(inputs["domain_id"])
    emb_tables = np.asarray(inputs["emb_tables"], dtype=np.float32)
    W1 = np.asarray(inputs["W1"], dtype=np.float32)
    g1 = np.asarray(inputs["g1"], dtype=np.float32)
    be1 = np.asarray(inputs["be1"], dtype=np.float32)
    W2 = np.asarray(inputs["W2"], dtype=np.float32)
    g2 = np.asarray(inputs["g2"], dtype=np.float32)
    be2 = np.asarray(inputs["be2"], dtype=np.float32)
    W3 = np.asarray(inputs["W3"], dtype=np.float32)
    b3 = np.asarray(inputs["b3"], dtype=np.float32)
    # b1/b2 cancel inside training-mode BatchNorm (constant shift).

    if _NC is None:
        _NC = _build()

    tab = np.ascontiguousarray(emb_tables.reshape(F * V, E).astype(bf))
    w1t = np.ascontiguousarray(
        W1.transpose(0, 2, 1).reshape(D, K1, P, H1)
        .transpose(0, 2, 1, 3).astype(bf))            # [D, P, K1, H1]
    w2t = np.ascontiguousarray(
        W2.transpose(0, 2, 1).reshape(D, K2, P, H2)
        .transpose(0, 2, 1, 3).astype(bf))            # [D, P, K2, H2]
    b3p = np.ascontiguousarray(b3.reshape(1, D))

    ids = feat_ids.astype(np.int64)
    in_maps = []
    for c in range(NCORES):
        idc = ids[c * BC:(c + 1) * BC]                # [BC, F]
        g = idc.reshape(NBT, P, F).transpose(1, 0, 2)
        g = g + (np.arange(F, dtype=np.int64) * V)[None, None, :]
        gidx = np.ascontiguousarray(g.reshape(P, NBT * F).astype(np.int32))
        in_maps.append({
            "tab": tab, "gidx": gidx,
            "w1t": w1t, "w2t": w2t,
            "g1": g1, "be1": be1, "g2": g2, "be2": be2,
            "w3": W3.astype(bf), "b3p": b3p,
        })

    res = run_bass_kernel_spmd(
        _NC, in_maps, core_ids=list(range(NCORES)), trace=bool(PROFILE))
    if PROFILE:
        LAST_EXEC_NS = res.exec_time_ns

    out_full = np.concatenate(
        [res.results[c]["out"] for c in range(NCORES)], axis=1)  # [D, B]
    final = out_full[domain_id.astype(np.int64), np.arange(B)]
    return final.astype(np.float32)
